# revision 18
# baseline (speedup 1.0000x reference)
"""Trainium2 Bass kernel for nn_DilateResUNetCLMemMLPPH.

Reference semantics (only image 0 matters):
  y_hat = argmax(labels[0, ::8, ::8, :], -1) flattened   [16384]
  y     = predicts[0] flattened                          [16384]
  per class c: stratified hard/easy first-k selection -> stream compaction
  X_[12288, 512] gathered rows of feats[0] (NHWC), zero-padded
  anchors = l2norm(mlp(X_)); y_ labels with IGNORE padding

Device strategy (8 cores):
  - pixels sharded 8x2048 across cores for the dense MLP (channel-major
    activations; layer3 emitted pixel-major via swapped matmul operands)
  - selection (argmax, masks, per-class segmented scans via
    tensor_tensor_scan + triangular-matmul row offsets, scalar count logic)
    is replicated on every core; per-core scatter destinations extracted
    with a one-hot matmul
  - each core scatters its normalized rows into a zero-initialized
    [12288, 256] output with indirect DMA (bounds_check drops unselected
    pixels); host sums the disjoint per-core outputs.
"""
import os
import sys

sys.path.insert(0, "/opt/trn_rl_repo")

import numpy as np

import concourse.bass as bass
from concourse import bacc
import concourse.tile as tile
from concourse import mybir
from concourse.tile import TileContext

f32 = mybir.dt.float32
i32 = mybir.dt.int32
ALU = mybir.AluOpType
AFT = mybir.ActivationFunctionType
AX = mybir.AxisListType

N_CORES = 8
H = W = 128
NPIX = H * W                     # 16384
C_IN = 512
C_MID = 256
PPC = NPIX // N_CORES            # 2048 pixels per core
ROWS_PC = 16                     # h-rows per core
N_OUT = 12288
IGNORE = 5
MV = [4096, 2048, 2048, 2048, 2048]
BASES = [0, 4096, 6144, 8192, 10240]
BIG = 1.0e6

# cons column layout
C_LAB = 0            # 5 * 128
C_YF = 640           # 128
C_UST = 768          # 128   strictly-lower prefix matrix U[k, m] = 1 if k < m
C_IDN = 896          # 128   identity
C_IOTA = 1024        # 96    local index within class block (flat = f*128 + p)
C_ONESCOL = 1120     # 1     ones on all partitions
C_ONESROW = 1121     # 128   ones on partition 0
C_B3ROW = 1249       # 256   b3 on partition 0
C_MV = 1505          # 5     mv per class, partition 0
C_BASEM1R = 1510     # 5     BASES[c] - 1, partition 0
C_BASEM1 = 1515      # 5     BASES[c] - 1, all partitions
NCONS = 1520


def build_program():
    nc = bacc.Bacc()

    x_d = nc.dram_tensor("x", [C_IN, PPC], f32, kind="ExternalInput")
    w1_d = nc.dram_tensor("w1t", [512, 512], f32, kind="ExternalInput")
    w2_d = nc.dram_tensor("w2t", [512, 256], f32, kind="ExternalInput")
    w3_d = nc.dram_tensor("w3t", [256, 256], f32, kind="ExternalInput")
    b1_d = nc.dram_tensor("b1c", [128, 4], f32, kind="ExternalInput")
    b2_d = nc.dram_tensor("b2c", [128, 2], f32, kind="ExternalInput")
    cons_d = nc.dram_tensor("cons", [128, NCONS], f32, kind="ExternalInput")
    oh_d = nc.dram_tensor("onehot", [128, ROWS_PC], f32, kind="ExternalInput")

    anch_d = nc.dram_tensor("anch", [N_OUT, C_MID], f32, kind="ExternalOutput")
    yout_d = nc.dram_tensor("yout", [96, 128], i32, kind="ExternalOutput")

    with TileContext(nc) as tc:
        with (
            tc.tile_pool(name="const", bufs=1) as constp,
            tc.tile_pool(name="sel", bufs=1) as sel,
            tc.tile_pool(name="mlp", bufs=1) as mlp,
            tc.tile_pool(name="work", bufs=3) as work,
            tc.tile_pool(name="ps", bufs=1, space="PSUM") as ps,
        ):
            # ---------------- loads ----------------
            cons = constp.tile([128, NCONS], f32, name="cons_sb")
            nc.sync.dma_start(out=cons[:], in_=cons_d[:, :])
            oh = constp.tile([128, ROWS_PC], f32, name="oh_sb")
            nc.sync.dma_start(out=oh[:], in_=oh_d[:, :])

            def cs(col, width):
                return cons[:, col:col + width]

            lab = lambda c: cs(C_LAB + c * 128, 128)
            yf = cs(C_YF, 128)

            # ---------------- selection (replicated) ----------------
            ct = []
            for c in range(5):
                t = sel.tile([128, 128], f32, name=f"ct{c}")
                nc.gpsimd.memset(t[:], float(c))
                ct.append(t)
            zerot = ct[0]

            best = sel.tile([128, 128], f32, name="best")
            nc.vector.tensor_copy(best[:], lab(0))
            Yt = sel.tile([128, 128], f32, name="Yt")
            nc.gpsimd.memset(Yt[:], 0.0)
            for c in range(1, 5):
                mt = work.tile([128, 128], i32, name="argmax_m", tag="argmax_m")
                nc.vector.tensor_tensor(out=mt[:], in0=lab(c), in1=best[:], op=ALU.is_gt)
                nc.vector.copy_predicated(best[:], mt[:], lab(c))
                nc.vector.copy_predicated(Yt[:], mt[:], ct[c][:])

            e1 = sel.tile([128, 640], f32, name="e1")
            e3 = sel.tile([128, 640], f32, name="e3")
            Em = sel.tile([128, 640], f32, name="Em")
            Hm = sel.tile([128, 640], f32, name="Hm")
            for c in range(5):
                nc.gpsimd.tensor_scalar(out=e1[:, c * 128:(c + 1) * 128], in0=Yt[:],
                                        scalar1=float(c), scalar2=None, op0=ALU.is_equal)
                nc.gpsimd.tensor_scalar(out=e3[:, c * 128:(c + 1) * 128], in0=yf,
                                        scalar1=float(c), scalar2=None, op0=ALU.is_equal)
            nc.vector.tensor_tensor(out=Em[:], in0=e1[:], in1=e3[:], op=ALU.mult)
            nc.vector.tensor_tensor(out=Hm[:], in0=e1[:], in1=Em[:], op=ALU.subtract)

            Hs = sel.tile([128, 640], f32, name="Hs")
            Es = sel.tile([128, 640], f32, name="Es")
            for c in range(5):
                sl = slice(c * 128, (c + 1) * 128)
                nc.vector.tensor_tensor_scan(out=Hs[:, sl], data0=Hm[:, sl], data1=zerot[:],
                                             initial=0.0, op0=ALU.add, op1=ALU.add)
                nc.vector.tensor_tensor_scan(out=Es[:, sl], data0=Em[:, sl], data1=zerot[:],
                                             initial=0.0, op0=ALU.add, op1=ALU.add)

            RS = sel.tile([128, 16], f32, name="RS")
            for c in range(5):
                sl = slice(c * 128, (c + 1) * 128)
                nc.vector.tensor_reduce(out=RS[:, c:c + 1], in_=Hm[:, sl], axis=AX.X, op=ALU.add)
                nc.vector.tensor_reduce(out=RS[:, 5 + c:6 + c], in_=Em[:, sl], axis=AX.X, op=ALU.add)

            offps = ps.tile([128, 16], f32, name="offps", tag="selps", bufs=2)
            nc.tensor.matmul(out=offps[:, 0:10], lhsT=cs(C_UST, 128), rhs=RS[:, 0:10],
                             start=True, stop=True)
            totps = ps.tile([1, 16], f32, name="totps", tag="selps2", bufs=1)
            nc.tensor.matmul(out=totps[:, 0:10], lhsT=cs(C_ONESCOL, 1), rhs=RS[:, 0:10],
                             start=True, stop=True)
            offsb = sel.tile([128, 10], f32, name="offsb")
            nc.vector.tensor_copy(offsb[:], offps[:, 0:10])
            tot = sel.tile([1, 10], f32, name="tot")
            nc.vector.tensor_copy(tot[:], totps[:, 0:10])

            # scalar chain on [1, 5] slices
            sc = sel.tile([1, 64], f32, name="scal")
            nh, ne = tot[:, 0:5], tot[:, 5:10]
            cnt, valid, mn = sc[:, 0:5], sc[:, 5:10], sc[:, 10:15]
            nsel, m2, sub2 = sc[:, 15:20], sc[:, 20:25], sc[:, 25:30]
            half, nh2, ne2 = sc[:, 30:35], sc[:, 35:40], sc[:, 40:45]
            c1, both, hk = sc[:, 45:50], sc[:, 50:55], sc[:, 55:60]

            nc.vector.tensor_tensor(out=cnt, in0=nh, in1=ne, op=ALU.add)
            nc.vector.tensor_scalar(out=valid, in0=cnt, scalar1=1.0, scalar2=None, op0=ALU.is_gt)
            nc.vector.tensor_tensor(out=mn, in0=cnt, in1=cons[0:1, C_MV:C_MV + 5], op=ALU.min)
            nc.vector.tensor_tensor(out=nsel, in0=mn, in1=valid, op=ALU.mult)
            # half = floor(nsel/2) via int parity (no mod on the DVE ISA)
            nseli = sel.tile([1, 8], i32, name="nseli")
            nc.vector.tensor_copy(nseli[:, 0:5], nsel)
            m2i = sel.tile([1, 8], i32, name="m2i")
            nc.vector.tensor_scalar(out=m2i[:, 0:5], in0=nseli[:, 0:5], scalar1=1,
                                    scalar2=None, op0=ALU.bitwise_and)
            nc.vector.tensor_copy(m2, m2i[:, 0:5])
            nc.vector.tensor_tensor(out=sub2, in0=nsel, in1=m2, op=ALU.subtract)
            nc.vector.tensor_scalar(out=half, in0=sub2, scalar1=0.5, scalar2=None, op0=ALU.mult)
            nc.vector.tensor_scalar(out=nh2, in0=nh, scalar1=2.0, scalar2=None, op0=ALU.mult)
            nc.vector.tensor_scalar(out=ne2, in0=ne, scalar1=2.0, scalar2=None, op0=ALU.mult)
            nc.vector.tensor_tensor(out=c1, in0=nh2, in1=nsel, op=ALU.is_ge)
            nc.vector.tensor_tensor(out=both, in0=ne2, in1=nsel, op=ALU.is_ge)
            nc.vector.tensor_tensor(out=both, in0=both, in1=c1, op=ALU.mult)
            # hk = nh + c1*(x - nh),  x = a + both*(half - a),  a = nsel - ne
            a_, hma, bx = sc[:, 25:30], sc[:, 20:25], sc[:, 35:40]  # reuse cols
            nc.vector.tensor_tensor(out=a_, in0=nsel, in1=ne, op=ALU.subtract)
            nc.vector.tensor_tensor(out=hma, in0=half, in1=a_, op=ALU.subtract)
            nc.vector.tensor_tensor(out=bx, in0=both, in1=hma, op=ALU.mult)
            xx = sc[:, 40:45]
            nc.vector.tensor_tensor(out=xx, in0=a_, in1=bx, op=ALU.add)
            xnh = sc[:, 30:35]
            nc.vector.tensor_tensor(out=xnh, in0=xx, in1=nh, op=ALU.subtract)
            c1x = sc[:, 50:55]  # reuse (both no longer needed)
            nc.vector.tensor_tensor(out=c1x, in0=c1, in1=xnh, op=ALU.mult)
            nc.vector.tensor_tensor(out=hk, in0=nh, in1=c1x, op=ALU.add)

            # brow: [hkadj | nseladj | nsel | hk]
            brow = sel.tile([1, 32], f32, name="brow")
            basem1r = cons[0:1, C_BASEM1R:C_BASEM1R + 5]
            nc.vector.tensor_tensor(out=brow[:, 0:5], in0=hk, in1=basem1r, op=ALU.add)
            nc.vector.tensor_tensor(out=brow[:, 5:10], in0=nsel, in1=basem1r, op=ALU.add)
            nc.vector.tensor_copy(brow[:, 10:15], nsel)
            nc.vector.tensor_copy(brow[:, 15:20], hk)

            bcps = ps.tile([128, 32], f32, name="bcps", tag="selps", bufs=2)
            nc.tensor.matmul(out=bcps[:, 0:20], lhsT=cons[0:1, C_ONESROW:C_ONESROW + 128],
                             rhs=brow[:, 0:20], start=True, stop=True)
            bc = sel.tile([128, 20], f32, name="bc")
            nc.vector.tensor_copy(bc[:], bcps[:, 0:20])

            offadj = sel.tile([128, 10], f32, name="offadj")
            basem1 = cs(C_BASEM1, 5)
            nc.vector.tensor_tensor(out=offadj[:, 0:5], in0=offsb[:, 0:5], in1=basem1, op=ALU.add)
            nc.vector.tensor_tensor(out=offadj[:, 5:10], in0=offsb[:, 5:10], in1=basem1, op=ALU.add)
            nc.vector.tensor_tensor(out=offadj[:, 5:10], in0=offadj[:, 5:10], in1=bc[:, 15:20], op=ALU.add)

            dh = sel.tile([128, 640], f32, name="dh")
            de = sel.tile([128, 640], f32, name="de")
            for c in range(5):
                sl = slice(c * 128, (c + 1) * 128)
                nc.scalar.activation(out=dh[:, sl], in_=Hs[:, sl], func=AFT.Identity,
                                     bias=offadj[:, c:c + 1])
                nc.scalar.activation(out=de[:, sl], in_=Es[:, sl], func=AFT.Identity,
                                     bias=offadj[:, 5 + c:6 + c])
            th = sel.tile([128, 640], f32, name="th")
            te = sel.tile([128, 640], f32, name="te")
            for c in range(5):
                sl = slice(c * 128, (c + 1) * 128)
                nc.gpsimd.tensor_scalar(out=th[:, sl], in0=dh[:, sl],
                                        scalar1=bc[:, c:c + 1], scalar2=None, op0=ALU.is_le)
                nc.gpsimd.tensor_scalar(out=te[:, sl], in0=de[:, sl],
                                        scalar1=bc[:, 5 + c:6 + c], scalar2=None, op0=ALU.is_le)
            hselI = sel.tile([128, 640], i32, name="hselI")
            eselI = sel.tile([128, 640], i32, name="eselI")
            nc.vector.tensor_tensor(out=hselI[:], in0=th[:], in1=Hm[:], op=ALU.mult)
            nc.vector.tensor_tensor(out=eselI[:], in0=te[:], in1=Em[:], op=ALU.mult)

            dtile = sel.tile([128, 128], f32, name="dtile")
            nc.gpsimd.memset(dtile[:], BIG)
            for c in range(5):
                sl = slice(c * 128, (c + 1) * 128)
                nc.vector.copy_predicated(dtile[:], hselI[:, sl], dh[:, sl])
                nc.vector.copy_predicated(dtile[:], eselI[:, sl], de[:, sl])

            dps = ps.tile([128, ROWS_PC], f32, name="dps", tag="selps", bufs=2)
            nc.tensor.matmul(out=dps[:], lhsT=dtile[:], rhs=oh[:], start=True, stop=True)
            dcore = sel.tile([128, ROWS_PC], i32, name="dcore")
            nc.vector.tensor_copy(dcore[:], dps[:])

            # ---------------- y_ output (replicated) ----------------
            yv = sel.tile([128, 96], f32, name="yv")
            nc.gpsimd.memset(yv[:], float(IGNORE))
            fo = 0
            for c in range(5):
                wdt = 32 if c == 0 else 16
                msk = work.tile([128, 32], i32, name="ymsk", tag="ymsk")
                nc.vector.tensor_scalar(out=msk[:, 0:wdt], in0=cs(C_IOTA + fo, wdt),
                                        scalar1=bc[:, 10 + c:11 + c], scalar2=None, op0=ALU.is_lt)
                nc.vector.copy_predicated(yv[:, fo:fo + wdt], msk[:, 0:wdt], ct[c][:, 0:wdt])
                fo += wdt
            yps = ps.tile([96, 128], f32, name="yps", tag="selps2", bufs=1)
            nc.tensor.transpose(out=yps[:], in_=yv[:], identity=cs(C_IDN, 128))
            yi = sel.tile([96, 128], i32, name="yi")
            nc.vector.tensor_copy(yi[:], yps[:])
            nc.sync.dma_start(out=yout_d[:, :], in_=yi[:])

            # ---------------- MLP ----------------
            w1sb = mlp.tile([128, 2048], f32, name="w1sb")
            nc.sync.dma_start(out=w1sb[:].rearrange("k (a m) -> k a m", a=4),
                              in_=w1_d[:, :].rearrange("(a k) m -> k a m", k=128))
            w2sb = mlp.tile([128, 1024], f32, name="w2sb")
            nc.sync.dma_start(out=w2sb[:].rearrange("k (a m) -> k a m", a=4),
                              in_=w2_d[:, :].rearrange("(a k) m -> k a m", k=128))
            w3sb = mlp.tile([128, 512], f32, name="w3sb")
            nc.sync.dma_start(out=w3sb[:].rearrange("k (a m) -> k a m", a=2),
                              in_=w3_d[:, :].rearrange("(a k) m -> k a m", k=128))
            b1c = mlp.tile([128, 4], f32, name="b1c_sb")
            nc.sync.dma_start(out=b1c[:], in_=b1_d[:, :])
            b2c = mlp.tile([128, 2], f32, name="b2c_sb")
            nc.sync.dma_start(out=b2c[:], in_=b2_d[:, :])
            # load x in 4 pixel-block chunks so layer-1 matmuls for block nb
            # start as soon as its 1MB lands (instead of after the full 4MB)
            xsb = mlp.tile([128, 8192], f32, name="xsb")
            for nb in range(4):
                nc.sync.dma_start(
                    out=xsb[:].rearrange("p (a n) -> p a n", a=4)[:, :, nb * 512:(nb + 1) * 512],
                    in_=x_d[:, nb * 512:(nb + 1) * 512].rearrange("(a p) n -> p a n", p=128))

            h1 = [mlp.tile([128, 2048], f32, name=f"h1_{m}") for m in range(4)]
            for nb in range(4):        # nb outer: consume x chunks as they arrive
                for m in range(4):
                    ps1 = ps.tile([128, 512], f32, name="ps1", tag="ps1", bufs=3)
                    for a in range(4):
                        nc.tensor.matmul(
                            out=ps1[:],
                            lhsT=w1sb[:, a * 512 + m * 128:a * 512 + (m + 1) * 128],
                            rhs=xsb[:, a * 2048 + nb * 512:a * 2048 + (nb + 1) * 512],
                            start=(a == 0), stop=(a == 3))
                    nc.scalar.activation(out=h1[m][:, nb * 512:(nb + 1) * 512], in_=ps1[:],
                                         func=AFT.Identity, bias=b1c[:, m:m + 1])
            for m in range(4):
                nc.vector.scalar_tensor_tensor(out=h1[m][:], in0=h1[m][:], scalar=0.2,
                                               in1=h1[m][:], op0=ALU.mult, op1=ALU.max)

            h2 = [mlp.tile([128, 2048], f32, name=f"h2_{m}") for m in range(2)]
            for m in range(2):
                for nb in range(4):
                    ps2 = ps.tile([128, 512], f32, name="ps2", tag="ps1", bufs=3)
                    for a in range(4):
                        nc.tensor.matmul(
                            out=ps2[:],
                            lhsT=w2sb[:, a * 256 + m * 128:a * 256 + (m + 1) * 128],
                            rhs=h1[a][:, nb * 512:(nb + 1) * 512],
                            start=(a == 0), stop=(a == 3))
                    nc.scalar.activation(out=h2[m][:, nb * 512:(nb + 1) * 512], in_=ps2[:],
                                         func=AFT.Identity, bias=b2c[:, m:m + 1])
                nc.vector.scalar_tensor_tensor(out=h2[m][:], in0=h2[m][:], scalar=0.2,
                                               in1=h2[m][:], op0=ALU.mult, op1=ALU.max)

            # layer 3 per pixel-tile, pixel-major out, fused bias matmul, scatter
            anc_all = mlp.tile([128, ROWS_PC * 256], f32, name="anc_all")
            for j in range(ROWS_PC):
                ps3 = ps.tile([128, 256], f32, name="ps3", tag="ps3", bufs=2)
                nc.tensor.matmul(out=ps3[:], lhsT=h2[0][:, j * 128:(j + 1) * 128],
                                 rhs=w3sb[:, 0:256], start=True, stop=False)
                nc.tensor.matmul(out=ps3[:], lhsT=h2[1][:, j * 128:(j + 1) * 128],
                                 rhs=w3sb[:, 256:512], start=False, stop=False)
                nc.tensor.matmul(out=ps3[:], lhsT=cons[0:1, C_ONESROW:C_ONESROW + 128],
                                 rhs=cons[0:1, C_B3ROW:C_B3ROW + 256], start=False, stop=True)
                junk = work.tile([128, 256], f32, name="junk", tag="junk")
                ssq = work.tile([128, 1], f32, name="ssq", tag="ssq")
                nc.scalar.activation(out=junk[:], in_=ps3[:], func=AFT.Square, accum_out=ssq[:])
                nr = work.tile([128, 1], f32, name="nr", tag="nr")
                nc.scalar.activation(out=nr[:], in_=ssq[:], func=AFT.Sqrt)
                nc.vector.tensor_scalar(out=nr[:], in0=nr[:], scalar1=1e-12, scalar2=None,
                                        op0=ALU.max)
                ri = work.tile([128, 1], f32, name="ri", tag="ri")
                nc.vector.reciprocal(ri[:], nr[:])
                nc.scalar.mul(anc_all[:, j * 256:(j + 1) * 256], ps3[:], ri[:, 0:1])

            # per-tile scatters: row (p, j) -> anch[dcore[p, j]]
            # (hardware SWDGE only honors [P, 1] offset tables)
            for j in range(ROWS_PC):
                nc.gpsimd.indirect_dma_start(
                    out=anch_d[:, :],
                    out_offset=bass.IndirectOffsetOnAxis(ap=dcore[:, j:j + 1], axis=0),
                    in_=anc_all[:, j * 256:(j + 1) * 256],
                    in_offset=None,
                    bounds_check=N_OUT - 1,
                    oob_is_err=False,
                )

    nc.finalize()
    return nc


_NC_CACHE = None


def _get_program():
    global _NC_CACHE
    if _NC_CACHE is None:
        _NC_CACHE = build_program()
    return _NC_CACHE


def build_host_inputs(feats, labels, predicts, w1, b1, w2, b2, w3, b3):
    """Build the 8 per-core input maps (host-side sharding / layout only)."""
    feats = np.asarray(feats, np.float32)
    labels = np.asarray(labels, np.float32)
    predicts = np.asarray(predicts)

    lab5 = np.ascontiguousarray(np.transpose(labels[0, ::8, ::8, :], (2, 0, 1)))
    yf = predicts[0].astype(np.float32)

    cons = np.zeros((128, NCONS), np.float32)
    cons[:, C_LAB:C_LAB + 640] = lab5.transpose(1, 0, 2).reshape(128, 640)
    cons[:, C_YF:C_YF + 128] = yf
    k = np.arange(128)
    cons[:, C_UST:C_UST + 128] = (k[:, None] < k[None, :]).astype(np.float32)
    cons[:, C_IDN:C_IDN + 128] = np.eye(128, dtype=np.float32)
    iota = np.zeros((128, 96), np.float32)
    fo = 0
    for c in range(5):
        wdt = 32 if c == 0 else 16
        loc = (np.arange(wdt)[None, :] * 128 + k[:, None]).astype(np.float32)
        iota[:, fo:fo + wdt] = loc
        fo += wdt
    cons[:, C_IOTA:C_IOTA + 96] = iota
    cons[:, C_ONESCOL] = 1.0
    cons[0, C_ONESROW:C_ONESROW + 128] = 1.0
    cons[0, C_B3ROW:C_B3ROW + 256] = np.asarray(b3, np.float32)
    cons[0, C_MV:C_MV + 5] = MV
    cons[0, C_BASEM1R:C_BASEM1R + 5] = np.array(BASES, np.float32) - 1.0
    cons[:, C_BASEM1:C_BASEM1 + 5] = (np.array(BASES, np.float32) - 1.0)[None, :]

    w1t = np.ascontiguousarray(np.asarray(w1, np.float32).T)   # [in, out]
    w2t = np.ascontiguousarray(np.asarray(w2, np.float32).T)   # [512, 256]
    w3t = np.ascontiguousarray(np.asarray(w3, np.float32).T)   # [256, 256]
    b1c = np.ascontiguousarray(np.asarray(b1, np.float32).reshape(4, 128).T)
    b2c = np.ascontiguousarray(np.asarray(b2, np.float32).reshape(2, 128).T)

    Fcm = feats[0].reshape(C_IN, NPIX)
    in_maps = []
    for kcore in range(N_CORES):
        oh = np.zeros((128, ROWS_PC), np.float32)
        for j in range(ROWS_PC):
            oh[kcore * ROWS_PC + j, j] = 1.0
        in_maps.append({
            "x": np.ascontiguousarray(Fcm[:, kcore * PPC:(kcore + 1) * PPC]),
            "w1t": w1t, "w2t": w2t, "w3t": w3t,
            "b1c": b1c, "b2c": b2c,
            "cons": cons, "onehot": oh,
        })
    return in_maps


RUN_KWARGS = {}


def kernel(**inputs):
    from concourse.bass_utils import run_bass_kernel_spmd

    nc = _get_program()
    in_maps = build_host_inputs(**inputs)
    res = run_bass_kernel_spmd(nc, in_maps, core_ids=list(range(N_CORES)), **RUN_KWARGS)
    kernel.last_results = res

    anchors = np.zeros((N_OUT, C_MID), np.float32)
    for r in res.results:
        anchors += np.asarray(r["anch"], np.float32)
    y = np.asarray(res.results[0]["yout"], np.int32).reshape(-1)

    # pad rows carry mlp(0) normalized; zero when biases are zero (as in the
    # spec), but handle the general case on host for robustness.
    b1 = np.asarray(inputs["b1"], np.float32)
    if np.any(b1) or np.any(np.asarray(inputs["b2"])) or np.any(np.asarray(inputs["b3"])):
        w1 = np.asarray(inputs["w1"], np.float32)
        w2 = np.asarray(inputs["w2"], np.float32)
        w3 = np.asarray(inputs["w3"], np.float32)
        b2 = np.asarray(inputs["b2"], np.float32)
        b3 = np.asarray(inputs["b3"], np.float32)
        lk = lambda v: np.where(v >= 0, v, 0.2 * v)
        h = lk(b1)
        h = lk(w2 @ h + b2)
        h = w3 @ h + b3
        pa = h / max(np.linalg.norm(h), 1e-12)
        anchors[y == IGNORE] = pa.astype(np.float32)

    return anchors, y


# revision 19
# speedup vs baseline: 198.2279x; 198.2279x over previous
"""Trainium2 Bass kernel for nn_DilateResUNetCLMemMLPPH.

Reference semantics (only image 0 matters):
  y_hat = argmax(labels[0, ::8, ::8, :], -1) flattened   [16384]
  y     = predicts[0] flattened                          [16384]
  per class c: stratified hard/easy first-k selection -> stream compaction
  X_[12288, 512] gathered rows of feats[0] (NHWC), zero-padded
  anchors = l2norm(mlp(X_)); y_ labels with IGNORE padding

Device strategy (8 cores):
  - pixels sharded 8x2048 across cores for the dense MLP (channel-major
    activations; layer3 emitted pixel-major via swapped matmul operands)
  - selection (argmax, masks, per-class segmented scans via
    tensor_tensor_scan + triangular-matmul row offsets, scalar count logic)
    is replicated on every core; per-core scatter destinations extracted
    with a one-hot matmul
  - each core scatters its normalized rows into a zero-initialized
    [12288, 256] output with indirect DMA (bounds_check drops unselected
    pixels); host sums the disjoint per-core outputs.
"""
import os
import sys

sys.path.insert(0, "/opt/trn_rl_repo")

import numpy as np

import concourse.bass as bass
from concourse import bacc
import concourse.tile as tile
from concourse import mybir
from concourse.tile import TileContext

f32 = mybir.dt.float32
i32 = mybir.dt.int32
ALU = mybir.AluOpType
AFT = mybir.ActivationFunctionType
AX = mybir.AxisListType

N_CORES = 8
H = W = 128
NPIX = H * W                     # 16384
C_IN = 512
C_MID = 256
PPC = NPIX // N_CORES            # 2048 pixels per core
ROWS_PC = 16                     # h-rows per core
N_OUT = 12288
IGNORE = 5
MV = [4096, 2048, 2048, 2048, 2048]
BASES = [0, 4096, 6144, 8192, 10240]
BIG = 1.0e6

# cons column layout
C_LAB = 0            # 5 * 128
C_YF = 640           # 128
C_UST = 768          # 128   strictly-lower prefix matrix U[k, m] = 1 if k < m
C_IDN = 896          # 128   identity
C_IOTA = 1024        # 96    local index within class block (flat = f*128 + p)
C_ONESCOL = 1120     # 1     ones on all partitions
C_ONESROW = 1121     # 128   ones on partition 0
C_B3ROW = 1249       # 256   b3 on partition 0
C_MV = 1505          # 5     mv per class, partition 0
C_BASEM1R = 1510     # 5     BASES[c] - 1, partition 0
C_BASEM1 = 1515      # 5     BASES[c] - 1, all partitions
NCONS = 1520


def build_program():
    nc = bacc.Bacc()

    x_d = nc.dram_tensor("x", [C_IN, PPC], f32, kind="ExternalInput")
    w1_d = nc.dram_tensor("w1t", [512, 512], f32, kind="ExternalInput")
    w2_d = nc.dram_tensor("w2t", [512, 256], f32, kind="ExternalInput")
    w3_d = nc.dram_tensor("w3t", [256, 256], f32, kind="ExternalInput")
    b1_d = nc.dram_tensor("b1c", [128, 4], f32, kind="ExternalInput")
    b2_d = nc.dram_tensor("b2c", [128, 2], f32, kind="ExternalInput")
    cons_d = nc.dram_tensor("cons", [128, NCONS], f32, kind="ExternalInput")
    oh_d = nc.dram_tensor("onehot", [128, ROWS_PC], f32, kind="ExternalInput")

    anch_d = nc.dram_tensor("anch", [N_OUT, C_MID], f32, kind="ExternalOutput")
    yout_d = nc.dram_tensor("yout", [96, 128], i32, kind="ExternalOutput")

    with TileContext(nc) as tc:
        with (
            tc.tile_pool(name="const", bufs=1) as constp,
            tc.tile_pool(name="sel", bufs=1) as sel,
            tc.tile_pool(name="mlp", bufs=1) as mlp,
            tc.tile_pool(name="work", bufs=3) as work,
            tc.tile_pool(name="ps", bufs=1, space="PSUM") as ps,
        ):
            # ---------------- loads ----------------
            cons = constp.tile([128, NCONS], f32, name="cons_sb")
            nc.sync.dma_start(out=cons[:], in_=cons_d[:, :])
            oh = constp.tile([128, ROWS_PC], f32, name="oh_sb")
            nc.sync.dma_start(out=oh[:], in_=oh_d[:, :])

            def cs(col, width):
                return cons[:, col:col + width]

            lab = lambda c: cs(C_LAB + c * 128, 128)
            yf = cs(C_YF, 128)

            # ---------------- selection (replicated) ----------------
            ct = []
            for c in range(5):
                t = sel.tile([128, 128], f32, name=f"ct{c}")
                nc.gpsimd.memset(t[:], float(c))
                ct.append(t)
            zerot = ct[0]

            best = sel.tile([128, 128], f32, name="best")
            nc.vector.tensor_copy(best[:], lab(0))
            Yt = sel.tile([128, 128], f32, name="Yt")
            nc.gpsimd.memset(Yt[:], 0.0)
            for c in range(1, 5):
                mt = work.tile([128, 128], i32, name="argmax_m", tag="argmax_m")
                nc.vector.tensor_tensor(out=mt[:], in0=lab(c), in1=best[:], op=ALU.is_gt)
                nc.vector.copy_predicated(best[:], mt[:], lab(c))
                nc.vector.copy_predicated(Yt[:], mt[:], ct[c][:])

            e1 = sel.tile([128, 640], f32, name="e1")
            e3 = sel.tile([128, 640], f32, name="e3")
            Em = sel.tile([128, 640], f32, name="Em")
            Hm = sel.tile([128, 640], f32, name="Hm")
            for c in range(5):
                nc.vector.tensor_scalar(out=e1[:, c * 128:(c + 1) * 128], in0=Yt[:],
                                        scalar1=float(c), scalar2=None, op0=ALU.is_equal)
                nc.vector.tensor_scalar(out=e3[:, c * 128:(c + 1) * 128], in0=yf,
                                        scalar1=float(c), scalar2=None, op0=ALU.is_equal)
            nc.vector.tensor_tensor(out=Em[:], in0=e1[:], in1=e3[:], op=ALU.mult)
            nc.vector.tensor_tensor(out=Hm[:], in0=e1[:], in1=Em[:], op=ALU.subtract)

            Hs = sel.tile([128, 640], f32, name="Hs")
            Es = sel.tile([128, 640], f32, name="Es")
            for c in range(5):
                sl = slice(c * 128, (c + 1) * 128)
                nc.vector.tensor_tensor_scan(out=Hs[:, sl], data0=Hm[:, sl], data1=zerot[:],
                                             initial=0.0, op0=ALU.add, op1=ALU.add)
                nc.vector.tensor_tensor_scan(out=Es[:, sl], data0=Em[:, sl], data1=zerot[:],
                                             initial=0.0, op0=ALU.add, op1=ALU.add)

            RS = sel.tile([128, 16], f32, name="RS")
            for c in range(5):
                sl = slice(c * 128, (c + 1) * 128)
                nc.vector.tensor_reduce(out=RS[:, c:c + 1], in_=Hm[:, sl], axis=AX.X, op=ALU.add)
                nc.vector.tensor_reduce(out=RS[:, 5 + c:6 + c], in_=Em[:, sl], axis=AX.X, op=ALU.add)

            offps = ps.tile([128, 16], f32, name="offps", tag="selps", bufs=2)
            nc.tensor.matmul(out=offps[:, 0:10], lhsT=cs(C_UST, 128), rhs=RS[:, 0:10],
                             start=True, stop=True)
            totps = ps.tile([1, 16], f32, name="totps", tag="selps2", bufs=1)
            nc.tensor.matmul(out=totps[:, 0:10], lhsT=cs(C_ONESCOL, 1), rhs=RS[:, 0:10],
                             start=True, stop=True)
            offsb = sel.tile([128, 10], f32, name="offsb")
            nc.vector.tensor_copy(offsb[:], offps[:, 0:10])
            tot = sel.tile([1, 10], f32, name="tot")
            nc.vector.tensor_copy(tot[:], totps[:, 0:10])

            # scalar chain on [1, 5] slices
            sc = sel.tile([1, 64], f32, name="scal")
            nh, ne = tot[:, 0:5], tot[:, 5:10]
            cnt, valid, mn = sc[:, 0:5], sc[:, 5:10], sc[:, 10:15]
            nsel, m2, sub2 = sc[:, 15:20], sc[:, 20:25], sc[:, 25:30]
            half, nh2, ne2 = sc[:, 30:35], sc[:, 35:40], sc[:, 40:45]
            c1, both, hk = sc[:, 45:50], sc[:, 50:55], sc[:, 55:60]

            nc.vector.tensor_tensor(out=cnt, in0=nh, in1=ne, op=ALU.add)
            nc.vector.tensor_scalar(out=valid, in0=cnt, scalar1=1.0, scalar2=None, op0=ALU.is_gt)
            nc.vector.tensor_tensor(out=mn, in0=cnt, in1=cons[0:1, C_MV:C_MV + 5], op=ALU.min)
            nc.vector.tensor_tensor(out=nsel, in0=mn, in1=valid, op=ALU.mult)
            # half = floor(nsel/2) via int parity (no mod on the DVE ISA)
            nseli = sel.tile([1, 8], i32, name="nseli")
            nc.vector.tensor_copy(nseli[:, 0:5], nsel)
            m2i = sel.tile([1, 8], i32, name="m2i")
            nc.vector.tensor_scalar(out=m2i[:, 0:5], in0=nseli[:, 0:5], scalar1=1,
                                    scalar2=None, op0=ALU.bitwise_and)
            nc.vector.tensor_copy(m2, m2i[:, 0:5])
            nc.vector.tensor_tensor(out=sub2, in0=nsel, in1=m2, op=ALU.subtract)
            nc.vector.tensor_scalar(out=half, in0=sub2, scalar1=0.5, scalar2=None, op0=ALU.mult)
            nc.vector.tensor_scalar(out=nh2, in0=nh, scalar1=2.0, scalar2=None, op0=ALU.mult)
            nc.vector.tensor_scalar(out=ne2, in0=ne, scalar1=2.0, scalar2=None, op0=ALU.mult)
            nc.vector.tensor_tensor(out=c1, in0=nh2, in1=nsel, op=ALU.is_ge)
            nc.vector.tensor_tensor(out=both, in0=ne2, in1=nsel, op=ALU.is_ge)
            nc.vector.tensor_tensor(out=both, in0=both, in1=c1, op=ALU.mult)
            # hk = nh + c1*(x - nh),  x = a + both*(half - a),  a = nsel - ne
            a_, hma, bx = sc[:, 25:30], sc[:, 20:25], sc[:, 35:40]  # reuse cols
            nc.vector.tensor_tensor(out=a_, in0=nsel, in1=ne, op=ALU.subtract)
            nc.vector.tensor_tensor(out=hma, in0=half, in1=a_, op=ALU.subtract)
            nc.vector.tensor_tensor(out=bx, in0=both, in1=hma, op=ALU.mult)
            xx = sc[:, 40:45]
            nc.vector.tensor_tensor(out=xx, in0=a_, in1=bx, op=ALU.add)
            xnh = sc[:, 30:35]
            nc.vector.tensor_tensor(out=xnh, in0=xx, in1=nh, op=ALU.subtract)
            c1x = sc[:, 50:55]  # reuse (both no longer needed)
            nc.vector.tensor_tensor(out=c1x, in0=c1, in1=xnh, op=ALU.mult)
            nc.vector.tensor_tensor(out=hk, in0=nh, in1=c1x, op=ALU.add)

            # brow: [hkadj | nseladj | nsel | hk]
            brow = sel.tile([1, 32], f32, name="brow")
            basem1r = cons[0:1, C_BASEM1R:C_BASEM1R + 5]
            nc.vector.tensor_tensor(out=brow[:, 0:5], in0=hk, in1=basem1r, op=ALU.add)
            nc.vector.tensor_tensor(out=brow[:, 5:10], in0=nsel, in1=basem1r, op=ALU.add)
            nc.vector.tensor_copy(brow[:, 10:15], nsel)
            nc.vector.tensor_copy(brow[:, 15:20], hk)

            bcps = ps.tile([128, 32], f32, name="bcps", tag="selps", bufs=2)
            nc.tensor.matmul(out=bcps[:, 0:20], lhsT=cons[0:1, C_ONESROW:C_ONESROW + 128],
                             rhs=brow[:, 0:20], start=True, stop=True)
            bc = sel.tile([128, 20], f32, name="bc")
            nc.vector.tensor_copy(bc[:], bcps[:, 0:20])

            offadj = sel.tile([128, 10], f32, name="offadj")
            basem1 = cs(C_BASEM1, 5)
            nc.vector.tensor_tensor(out=offadj[:, 0:5], in0=offsb[:, 0:5], in1=basem1, op=ALU.add)
            nc.vector.tensor_tensor(out=offadj[:, 5:10], in0=offsb[:, 5:10], in1=basem1, op=ALU.add)
            nc.vector.tensor_tensor(out=offadj[:, 5:10], in0=offadj[:, 5:10], in1=bc[:, 15:20], op=ALU.add)

            dh = sel.tile([128, 640], f32, name="dh")
            de = sel.tile([128, 640], f32, name="de")
            for c in range(5):
                sl = slice(c * 128, (c + 1) * 128)
                nc.scalar.activation(out=dh[:, sl], in_=Hs[:, sl], func=AFT.Identity,
                                     bias=offadj[:, c:c + 1])
                nc.scalar.activation(out=de[:, sl], in_=Es[:, sl], func=AFT.Identity,
                                     bias=offadj[:, 5 + c:6 + c])
            th = sel.tile([128, 640], f32, name="th")
            te = sel.tile([128, 640], f32, name="te")
            for c in range(5):
                sl = slice(c * 128, (c + 1) * 128)
                nc.vector.tensor_scalar(out=th[:, sl], in0=dh[:, sl],
                                        scalar1=bc[:, c:c + 1], scalar2=None, op0=ALU.is_le)
                nc.vector.tensor_scalar(out=te[:, sl], in0=de[:, sl],
                                        scalar1=bc[:, 5 + c:6 + c], scalar2=None, op0=ALU.is_le)
            hselI = sel.tile([128, 640], i32, name="hselI")
            eselI = sel.tile([128, 640], i32, name="eselI")
            nc.vector.tensor_tensor(out=hselI[:], in0=th[:], in1=Hm[:], op=ALU.mult)
            nc.vector.tensor_tensor(out=eselI[:], in0=te[:], in1=Em[:], op=ALU.mult)

            dtile = sel.tile([128, 128], f32, name="dtile")
            nc.gpsimd.memset(dtile[:], BIG)
            for c in range(5):
                sl = slice(c * 128, (c + 1) * 128)
                nc.vector.copy_predicated(dtile[:], hselI[:, sl], dh[:, sl])
                nc.vector.copy_predicated(dtile[:], eselI[:, sl], de[:, sl])

            dps = ps.tile([128, ROWS_PC], f32, name="dps", tag="selps", bufs=2)
            nc.tensor.matmul(out=dps[:], lhsT=dtile[:], rhs=oh[:], start=True, stop=True)
            dcore = sel.tile([128, ROWS_PC], i32, name="dcore")
            nc.vector.tensor_copy(dcore[:], dps[:])

            # ---------------- y_ output (replicated) ----------------
            yv = sel.tile([128, 96], f32, name="yv")
            nc.gpsimd.memset(yv[:], float(IGNORE))
            fo = 0
            for c in range(5):
                wdt = 32 if c == 0 else 16
                msk = work.tile([128, 32], i32, name="ymsk", tag="ymsk")
                nc.vector.tensor_scalar(out=msk[:, 0:wdt], in0=cs(C_IOTA + fo, wdt),
                                        scalar1=bc[:, 10 + c:11 + c], scalar2=None, op0=ALU.is_lt)
                nc.vector.copy_predicated(yv[:, fo:fo + wdt], msk[:, 0:wdt], ct[c][:, 0:wdt])
                fo += wdt
            yps = ps.tile([96, 128], f32, name="yps", tag="selps2", bufs=1)
            nc.tensor.transpose(out=yps[:], in_=yv[:], identity=cs(C_IDN, 128))
            yi = sel.tile([96, 128], i32, name="yi")
            nc.vector.tensor_copy(yi[:], yps[:])
            nc.sync.dma_start(out=yout_d[:, :], in_=yi[:])

            # ---------------- MLP ----------------
            w1sb = mlp.tile([128, 2048], f32, name="w1sb")
            nc.sync.dma_start(out=w1sb[:].rearrange("k (a m) -> k a m", a=4),
                              in_=w1_d[:, :].rearrange("(a k) m -> k a m", k=128))
            w2sb = mlp.tile([128, 1024], f32, name="w2sb")
            nc.sync.dma_start(out=w2sb[:].rearrange("k (a m) -> k a m", a=4),
                              in_=w2_d[:, :].rearrange("(a k) m -> k a m", k=128))
            w3sb = mlp.tile([128, 512], f32, name="w3sb")
            nc.sync.dma_start(out=w3sb[:].rearrange("k (a m) -> k a m", a=2),
                              in_=w3_d[:, :].rearrange("(a k) m -> k a m", k=128))
            b1c = mlp.tile([128, 4], f32, name="b1c_sb")
            nc.sync.dma_start(out=b1c[:], in_=b1_d[:, :])
            b2c = mlp.tile([128, 2], f32, name="b2c_sb")
            nc.sync.dma_start(out=b2c[:], in_=b2_d[:, :])
            xsb = mlp.tile([128, 8192], f32, name="xsb")
            nc.sync.dma_start(out=xsb[:].rearrange("p (a n) -> p a n", a=4),
                              in_=x_d[:, :].rearrange("(a p) n -> p a n", p=128))

            h1 = [mlp.tile([128, 2048], f32, name=f"h1_{m}") for m in range(4)]
            for m in range(4):
                for nb in range(4):
                    ps1 = ps.tile([128, 512], f32, name="ps1", tag="ps1", bufs=3)
                    for a in range(4):
                        nc.tensor.matmul(
                            out=ps1[:],
                            lhsT=w1sb[:, a * 512 + m * 128:a * 512 + (m + 1) * 128],
                            rhs=xsb[:, a * 2048 + nb * 512:a * 2048 + (nb + 1) * 512],
                            start=(a == 0), stop=(a == 3))
                    nc.scalar.activation(out=h1[m][:, nb * 512:(nb + 1) * 512], in_=ps1[:],
                                         func=AFT.Identity, bias=b1c[:, m:m + 1])
                nc.vector.scalar_tensor_tensor(out=h1[m][:], in0=h1[m][:], scalar=0.2,
                                               in1=h1[m][:], op0=ALU.mult, op1=ALU.max)

            h2 = [mlp.tile([128, 2048], f32, name=f"h2_{m}") for m in range(2)]
            for m in range(2):
                for nb in range(4):
                    ps2 = ps.tile([128, 512], f32, name="ps2", tag="ps1", bufs=3)
                    for a in range(4):
                        nc.tensor.matmul(
                            out=ps2[:],
                            lhsT=w2sb[:, a * 256 + m * 128:a * 256 + (m + 1) * 128],
                            rhs=h1[a][:, nb * 512:(nb + 1) * 512],
                            start=(a == 0), stop=(a == 3))
                    nc.scalar.activation(out=h2[m][:, nb * 512:(nb + 1) * 512], in_=ps2[:],
                                         func=AFT.Identity, bias=b2c[:, m:m + 1])
                nc.vector.scalar_tensor_tensor(out=h2[m][:], in0=h2[m][:], scalar=0.2,
                                               in1=h2[m][:], op0=ALU.mult, op1=ALU.max)

            # layer 3 per pixel-tile, pixel-major out, fused bias matmul, scatter
            anc_all = mlp.tile([128, ROWS_PC * 256], f32, name="anc_all")
            for j in range(ROWS_PC):
                ps3 = ps.tile([128, 256], f32, name="ps3", tag="ps3", bufs=2)
                nc.tensor.matmul(out=ps3[:], lhsT=h2[0][:, j * 128:(j + 1) * 128],
                                 rhs=w3sb[:, 0:256], start=True, stop=False)
                nc.tensor.matmul(out=ps3[:], lhsT=h2[1][:, j * 128:(j + 1) * 128],
                                 rhs=w3sb[:, 256:512], start=False, stop=False)
                nc.tensor.matmul(out=ps3[:], lhsT=cons[0:1, C_ONESROW:C_ONESROW + 128],
                                 rhs=cons[0:1, C_B3ROW:C_B3ROW + 256], start=False, stop=True)
                junk = work.tile([128, 256], f32, name="junk", tag="junk")
                ssq = work.tile([128, 1], f32, name="ssq", tag="ssq")
                nc.scalar.activation(out=junk[:], in_=ps3[:], func=AFT.Square, accum_out=ssq[:])
                nr = work.tile([128, 1], f32, name="nr", tag="nr")
                nc.scalar.activation(out=nr[:], in_=ssq[:], func=AFT.Sqrt)
                nc.vector.tensor_scalar(out=nr[:], in0=nr[:], scalar1=1e-12, scalar2=None,
                                        op0=ALU.max)
                ri = work.tile([128, 1], f32, name="ri", tag="ri")
                nc.vector.reciprocal(ri[:], nr[:])
                nc.vector.tensor_scalar(out=anc_all[:, j * 256:(j + 1) * 256], in0=ps3[:],
                                        scalar1=ri[:, 0:1], scalar2=None, op0=ALU.mult)

            # per-tile scatters: row (p, j) -> anch[dcore[p, j]]
            # (hardware SWDGE only honors [P, 1] offset tables)
            for j in range(ROWS_PC):
                nc.gpsimd.indirect_dma_start(
                    out=anch_d[:, :],
                    out_offset=bass.IndirectOffsetOnAxis(ap=dcore[:, j:j + 1], axis=0),
                    in_=anc_all[:, j * 256:(j + 1) * 256],
                    in_offset=None,
                    bounds_check=N_OUT - 1,
                    oob_is_err=False,
                )

    nc.finalize()
    return nc


_NC_CACHE = None


def _get_program():
    global _NC_CACHE
    if _NC_CACHE is None:
        _NC_CACHE = build_program()
    return _NC_CACHE


def build_host_inputs(feats, labels, predicts, w1, b1, w2, b2, w3, b3):
    """Build the 8 per-core input maps (host-side sharding / layout only)."""
    feats = np.asarray(feats, np.float32)
    labels = np.asarray(labels, np.float32)
    predicts = np.asarray(predicts)

    lab5 = np.ascontiguousarray(np.transpose(labels[0, ::8, ::8, :], (2, 0, 1)))
    yf = predicts[0].astype(np.float32)

    cons = np.zeros((128, NCONS), np.float32)
    cons[:, C_LAB:C_LAB + 640] = lab5.transpose(1, 0, 2).reshape(128, 640)
    cons[:, C_YF:C_YF + 128] = yf
    k = np.arange(128)
    cons[:, C_UST:C_UST + 128] = (k[:, None] < k[None, :]).astype(np.float32)
    cons[:, C_IDN:C_IDN + 128] = np.eye(128, dtype=np.float32)
    iota = np.zeros((128, 96), np.float32)
    fo = 0
    for c in range(5):
        wdt = 32 if c == 0 else 16
        loc = (np.arange(wdt)[None, :] * 128 + k[:, None]).astype(np.float32)
        iota[:, fo:fo + wdt] = loc
        fo += wdt
    cons[:, C_IOTA:C_IOTA + 96] = iota
    cons[:, C_ONESCOL] = 1.0
    cons[0, C_ONESROW:C_ONESROW + 128] = 1.0
    cons[0, C_B3ROW:C_B3ROW + 256] = np.asarray(b3, np.float32)
    cons[0, C_MV:C_MV + 5] = MV
    cons[0, C_BASEM1R:C_BASEM1R + 5] = np.array(BASES, np.float32) - 1.0
    cons[:, C_BASEM1:C_BASEM1 + 5] = (np.array(BASES, np.float32) - 1.0)[None, :]

    w1t = np.ascontiguousarray(np.asarray(w1, np.float32).T)   # [in, out]
    w2t = np.ascontiguousarray(np.asarray(w2, np.float32).T)   # [512, 256]
    w3t = np.ascontiguousarray(np.asarray(w3, np.float32).T)   # [256, 256]
    b1c = np.ascontiguousarray(np.asarray(b1, np.float32).reshape(4, 128).T)
    b2c = np.ascontiguousarray(np.asarray(b2, np.float32).reshape(2, 128).T)

    Fcm = feats[0].reshape(C_IN, NPIX)
    in_maps = []
    for kcore in range(N_CORES):
        oh = np.zeros((128, ROWS_PC), np.float32)
        for j in range(ROWS_PC):
            oh[kcore * ROWS_PC + j, j] = 1.0
        in_maps.append({
            "x": np.ascontiguousarray(Fcm[:, kcore * PPC:(kcore + 1) * PPC]),
            "w1t": w1t, "w2t": w2t, "w3t": w3t,
            "b1c": b1c, "b2c": b2c,
            "cons": cons, "onehot": oh,
        })
    return in_maps


RUN_KWARGS = {}


def kernel(**inputs):
    from concourse.bass_utils import run_bass_kernel_spmd

    nc = _get_program()
    in_maps = build_host_inputs(**inputs)
    res = run_bass_kernel_spmd(nc, in_maps, core_ids=list(range(N_CORES)), **RUN_KWARGS)
    kernel.last_results = res

    anchors = np.zeros((N_OUT, C_MID), np.float32)
    for r in res.results:
        anchors += np.asarray(r["anch"], np.float32)
    y = np.asarray(res.results[0]["yout"], np.int32).reshape(-1)

    # pad rows carry mlp(0) normalized; zero when biases are zero (as in the
    # spec), but handle the general case on host for robustness.
    b1 = np.asarray(inputs["b1"], np.float32)
    if np.any(b1) or np.any(np.asarray(inputs["b2"])) or np.any(np.asarray(inputs["b3"])):
        w1 = np.asarray(inputs["w1"], np.float32)
        w2 = np.asarray(inputs["w2"], np.float32)
        w3 = np.asarray(inputs["w3"], np.float32)
        b2 = np.asarray(inputs["b2"], np.float32)
        b3 = np.asarray(inputs["b3"], np.float32)
        lk = lambda v: np.where(v >= 0, v, 0.2 * v)
        h = lk(b1)
        h = lk(w2 @ h + b2)
        h = w3 @ h + b3
        pa = h / max(np.linalg.norm(h), 1e-12)
        anchors[y == IGNORE] = pa.astype(np.float32)

    return anchors, y
